# revision 1
# baseline (speedup 1.0000x reference)
"""Trainium2 Bass kernel for nn_EfficientTransformer_57002805952728.

Sharding: 8 cores = (batch, sequence-half) pairs. Each core processes 4096
own tokens + a 1024-token left halo (redundant compute; block-local attention
only looks back one 256-token window per layer, so after 4 layers the halo
absorbs all cross-boundary influence). The globally-first window's
"previous block" masking is handled by a per-token `valid` input (1e-30 on
dead halo) that multiplies v and forms the softmax-denominator column, so
the same NEFF runs SPMD on all 8 cores with differences only in input data.

On-chip layout is channel-major (hT[c, t]) so every matmul contracts over
partitions without transposes; attention computes scoresT = kT-slice @ qT
with 2 heads packed into the PE array via partition-offset row tiling, and
att@v with an extra "valid" lhsT column so the softmax denominator falls out
of the same accumulation. LayerNorm gains/biases are folded into the next
matmul's weights host-side; LN stats use ones-vector matmuls (cross-partition
sums) and a DMA partition-broadcast for the per-token scales.
"""

import numpy as np
import ml_dtypes

import concourse.bacc as bacc
import concourse.bass as bass
import concourse.tile as tile
import concourse.mybir as mybir
from concourse import bass_utils

BF16 = ml_dtypes.bfloat16
F32 = np.float32

B, S, F = 4, 8192, 32
D, H, L = 512, 8, 4
FD = 128
WIN = 256
CF = 4
HD = D // H
EPS = 1e-5

HALO = 4 * WIN          # 1024
T = S // 2 + HALO       # 5120 tokens per core (incl. halo)
CHUNK = 512
TOUT = (S // 2) // CF   # 1024 output rows per core
N_CORES = 8

AF = mybir.ActivationFunctionType


# ---------------------------------------------------------------------------
# Bass program
# ---------------------------------------------------------------------------

def build_nc(t_tokens=T, debug=False):
    op = mybir.AluOpType
    t = t_tokens
    nch = t // CHUNK
    tout = (t - HALO) // CF

    nc = bacc.Bacc("TRN2", target_bir_lowering=False, debug=False,
                   num_devices=N_CORES)
    bf = mybir.dt.bfloat16
    f32 = mybir.dt.float32

    d_xT = nc.dram_tensor("xT", [F, t], bf, kind="ExternalInput").ap()
    d_teT = nc.dram_tensor("teT", [FD, t], bf, kind="ExternalInput").ap()
    d_validP = nc.dram_tensor("validP", [128, t // 128], f32, kind="ExternalInput").ap()
    d_Wq = nc.dram_tensor("Wq", [L, 4, 128, 512], bf, kind="ExternalInput").ap()
    d_Wk = nc.dram_tensor("Wk", [L, 4, 128, 512], bf, kind="ExternalInput").ap()
    d_Wv = nc.dram_tensor("Wv", [L, 4, 128, 512], bf, kind="ExternalInput").ap()
    d_Wo = nc.dram_tensor("Wo", [L, 4, 128, 512], bf, kind="ExternalInput").ap()
    d_Wg = nc.dram_tensor("Wg", [L, 8, 128, 512], bf, kind="ExternalInput").ap()
    d_Wu = nc.dram_tensor("Wu", [L, 8, 128, 512], bf, kind="ExternalInput").ap()
    d_lbias = nc.dram_tensor("lbias", [L, 128, 24], f32, kind="ExternalInput").ap()
    d_Wfeat = nc.dram_tensor("Wfeat", [F, FD], bf, kind="ExternalInput").ap()
    d_We = nc.dram_tensor("We", [2, FD, D], bf, kind="ExternalInput").ap()
    d_bemb = nc.dram_tensor("bemb", [128, 4], f32, kind="ExternalInput").ap()
    d_Wconv = nc.dram_tensor("Wconv", [CF, 4, 128, 512], bf, kind="ExternalInput").ap()
    # fbias: col 0-3 conv_b', col 4-7 cn_g, col 8-11 cn_b (each [512] as [128,4])
    d_fbias = nc.dram_tensor("fbias", [128, 12], f32, kind="ExternalInput").ap()
    d_Wout = nc.dram_tensor("Wout", [4, 128, F], bf, kind="ExternalInput").ap()
    d_bout = nc.dram_tensor("bout", [F, 1], f32, kind="ExternalInput").ap()
    d_outT = nc.dram_tensor("outT", [F, tout], f32, kind="ExternalOutput").ap()
    d_dbg = {}
    if debug:
        d_dbg["emb"] = nc.dram_tensor("dbg_emb", [4, 128, t], bf, kind="ExternalOutput").ap()
        for l in range(L):
            d_dbg[f"h{l}"] = nc.dram_tensor(f"dbg_h{l}", [4, 128, t], bf, kind="ExternalOutput").ap()
            d_dbg[f"m{l}"] = nc.dram_tensor(f"dbg_m{l}", [4, 128, t], bf, kind="ExternalOutput").ap()

    with tile.TileContext(nc) as tc:
        const = tc.alloc_tile_pool(name="const", bufs=1)
        state = tc.alloc_tile_pool(name="state", bufs=1)
        p_mm = tc.alloc_tile_pool(name="p_mm", bufs=3, space="PSUM")
        p_sc = tc.alloc_tile_pool(name="p_sc", bufs=2, space="PSUM")
        p_av = tc.alloc_tile_pool(name="p_av", bufs=3, space="PSUM")

        # constants
        validP = const.tile([128, t // 128], f32, name="validP")
        nc.sync.dma_start(out=validP, in_=d_validP)
        ones_col = const.tile([128, 1], bf, name="ones_col")
        nc.vector.memset(ones_col, 1.0)
        eps_col = const.tile([128, 1], f32, name="eps_col")
        nc.vector.memset(eps_col, EPS)
        ones_row = const.tile([1, 64], f32, name="ones_row")
        nc.vector.memset(ones_row, 1.0)
        zkT = const.tile([128, WIN], bf, name="zkT")
        nc.vector.memset(zkT, 0.0)
        zvE = const.tile([128, H, HD + 1], bf, name="zvE")
        nc.vector.memset(zvE, 0.0)
        bemb = const.tile([128, 4], f32, name="bemb")
        nc.sync.dma_start(out=bemb, in_=d_bemb)
        Wfeat = const.tile([F, FD], bf, name="Wfeat")
        nc.sync.dma_start(out=Wfeat, in_=d_Wfeat)
        We = const.tile([FD, 2, D], bf, name="We")
        nc.sync.dma_start(out=We, in_=d_We.rearrange("e p d -> p e d"))

        # persistent activations (channel-major, 4 chunks of 128 channels)
        hT = [state.tile([128, t], bf, name=f"hT{c}") for c in range(4)]
        memT = [state.tile([128, t], bf, name=f"memT{c}") for c in range(4)]
        for c in range(4):
            nc.vector.memset(memT[c], 0.0)

        work = tc.alloc_tile_pool(name="work", bufs=2)

        # ------------------------------------------------------------------
        # embedding:  hT = We1.T @ (Wfeat.T @ xT) + We2.T @ teT + bemb'
        # ------------------------------------------------------------------
        for i in range(nch):
            sl = bass.ts(i, CHUNK)
            xTc = work.tile([F, CHUNK], bf, name="xTc", tag="gate_e", bufs=2)
            nc.sync.dma_start(out=xTc, in_=d_xT[:, sl])
            teTc = work.tile([FD, CHUNK], bf, name="teTc", tag="gate", bufs=2)
            nc.sync.dma_start(out=teTc, in_=d_teT[:, sl])
            fe_ps = p_mm.tile([128, CHUNK], f32, name="fe_ps", tag="mm")
            nc.tensor.matmul(fe_ps, Wfeat, xTc, start=True, stop=True)
            feT = work.tile([128, CHUNK], bf, name="feT", tag="tmp", bufs=2)
            nc.scalar.copy(feT, fe_ps)
            for dd in range(4):
                h_ps = p_mm.tile([128, CHUNK], f32, name="h_ps", tag="mm")
                nc.tensor.matmul(h_ps, We[:, 0, bass.ts(dd, 128)], feT,
                                 start=True, stop=False)
                nc.tensor.matmul(h_ps, We[:, 1, bass.ts(dd, 128)], teTc,
                                 start=False, stop=True)
                nc.scalar.activation(hT[dd][:, sl], h_ps, AF.Identity,
                                     bias=bemb[:, dd:dd + 1])

        if debug:
            for c in range(4):
                nc.sync.dma_start(out=d_dbg["emb"][c], in_=hT[c])

        # ------------------------------------------------------------------
        # transformer layers
        # ------------------------------------------------------------------
        wpool = tc.alloc_tile_pool(name="wpool", bufs=1)

        for l in range(L):
            Wq = wpool.tile([128, 4, 512], bf, name="Wq", tag="wq")
            Wk = wpool.tile([128, 4, 512], bf, name="Wk", tag="wk")
            Wv = wpool.tile([128, 4, 512], bf, name="Wv", tag="wv")
            Wo = wpool.tile([128, 4, 512], bf, name="Wo", tag="wo")
            Wg = wpool.tile([128, 8, 512], bf, name="Wg", tag="wg")
            Wu = wpool.tile([128, 8, 512], bf, name="Wu", tag="wu")
            lb = wpool.tile([128, 24], f32, name="lb", tag="lb")
            nc.sync.dma_start(out=Wq, in_=d_Wq[l].rearrange("c p d -> p c d"))
            nc.sync.dma_start(out=Wk, in_=d_Wk[l].rearrange("c p d -> p c d"))
            nc.sync.dma_start(out=Wv, in_=d_Wv[l].rearrange("c p d -> p c d"))
            nc.sync.dma_start(out=Wo, in_=d_Wo[l].rearrange("c p d -> p c d"))
            nc.sync.dma_start(out=Wg, in_=d_Wg[l].rearrange("c p d -> p c d"))
            nc.sync.dma_start(out=Wu, in_=d_Wu[l].rearrange("c p d -> p c d"))
            nc.sync.dma_start(out=lb, in_=d_lbias[l])

            # Layer l only needs valid queries on [HALO - 256*(L-l) ... t);
            # chunk-rounded starts skip fully-dead halo chunks. When skipping,
            # a small prologue computes just the k/v of the preceding block.
            cs = ([0, 1, 1, 2][l] if (nch == 10 and L == 4) else 0)
            if cs == 0:
                prev_k = None   # 4 pair tiles [128, 512]
                prev_v = None   # 4 t-group tiles [128, H, 65]
            else:
                blk0 = 2 * cs - 1
                tsl = bass.ts(blk0, WIN)
                pk = []
                for dd in range(4):
                    k_ps = p_mm.tile([128, 512], f32, name="kp_ps", tag="mm")
                    for c in range(4):
                        nc.tensor.matmul(k_ps[:, 0:WIN], Wk[:, c, bass.ts(dd, 128)],
                                         hT[c][:, tsl], start=(c == 0), stop=(c == 3))
                    ktp = work.tile([128, CHUNK], bf, name="ktp", tag="kT", bufs=12)
                    nc.scalar.activation(ktp[:, WIN:2 * WIN], k_ps[:, 0:WIN],
                                         AF.Identity, bias=lb[:, 4 + dd:5 + dd])
                    pk.append(ktp)
                pv = [None, None]
                for tg in (2, 3):
                    tok0 = (cs - 1) * CHUNK + tg * 128
                    v_ps = p_mm.tile([128, 512], f32, name="vp_ps", tag="mm")
                    for c in range(4):
                        nc.tensor.matmul(v_ps, hT[c][:, tok0:tok0 + 128],
                                         Wv[:, c, :], start=(c == 0), stop=(c == 3))
                    vep = work.tile([128, H, HD + 1], bf, name="vep", tag="vE", bufs=12)
                    gcol = (cs - 1) * 4 + tg
                    nc.vector.tensor_scalar(
                        out=vep[:, :, 0:HD],
                        in0=v_ps.rearrange("p (h d) -> p h d", h=H),
                        scalar1=validP[:, gcol:gcol + 1], scalar2=None,
                        op0=op.mult)
                    nc.vector.tensor_copy(
                        out=vep[:, :, HD:HD + 1],
                        in_=validP[:, gcol:gcol + 1].broadcast_to([128, H, 1]))
                    pv.append(vep)
                prev_k = pk
                prev_v = pv

            def phase_a(i):
                sl = bass.ts(i, CHUNK)
                qT, kT, vE = [], [], []
                for dd in range(4):
                    q_ps = p_mm.tile([128, CHUNK], f32, name="q_ps", tag="mm")
                    for c in range(4):
                        nc.tensor.matmul(q_ps, Wq[:, c, bass.ts(dd, 128)],
                                         hT[c][:, sl], start=(c == 0), stop=(c == 3))
                    qt = work.tile([128, CHUNK], bf, name="qt", tag="qT", bufs=8)
                    nc.vector.tensor_scalar(out=qt, in0=q_ps, scalar1=lb[:, dd:dd + 1],
                                            scalar2=None, op0=op.add)
                    qT.append(qt)
                for dd in range(4):
                    k_ps = p_mm.tile([128, CHUNK], f32, name="k_ps", tag="mm")
                    for c in range(4):
                        nc.tensor.matmul(k_ps, Wk[:, c, bass.ts(dd, 128)],
                                         hT[c][:, sl], start=(c == 0), stop=(c == 3))
                    kt = work.tile([128, CHUNK], bf, name="kt", tag="kT", bufs=12)
                    nc.vector.tensor_scalar(out=kt, in0=k_ps, scalar1=lb[:, 4 + dd:5 + dd],
                                            scalar2=None, op0=op.add)
                    kT.append(kt)
                for tg in range(4):
                    v_ps = p_mm.tile([128, 512], f32, name="v_ps", tag="mm")
                    for c in range(4):
                        nc.tensor.matmul(v_ps, hT[c][:, i * CHUNK + tg * 128:
                                                     i * CHUNK + (tg + 1) * 128],
                                         Wv[:, c, :], start=(c == 0), stop=(c == 3))
                    ve = work.tile([128, H, HD + 1], bf, name="ve", tag="vE", bufs=12)
                    gcol = i * 4 + tg
                    nc.vector.tensor_scalar(
                        out=ve[:, :, 0:HD],
                        in0=v_ps.rearrange("p (h d) -> p h d", h=H),
                        scalar1=validP[:, gcol:gcol + 1], scalar2=None,
                        op0=op.mult)
                    nc.vector.tensor_copy(
                        out=ve[:, :, HD:HD + 1],
                        in_=validP[:, gcol:gcol + 1].broadcast_to([128, H, 1]))
                    vE.append(ve)
                return qT, kT, vE

            def phase_b(i, st_i, st_prev):
                qT, kT, vE = st_i
                avT = [work.tile([128, CHUNK], bf, name="avt", tag="avT", bufs=5)
                       for _ in range(4)]
                for b2 in range(2):
                    qsl = bass.ts(b2, WIN)
                    if b2 == 0:
                        if st_prev is None:
                            kprev = [(zkT, 0)] * 4
                            vprev = [zvE, zvE]
                        else:
                            kprev = [(st_prev[0][p], WIN) for p in range(4)]
                            vprev = [st_prev[1][2], st_prev[1][3]]
                    else:
                        kprev = [(kT[p], 0) for p in range(4)]
                        vprev = [vE[0], vE[1]]
                    kcur = [(kT[p], b2 * WIN) for p in range(4)]
                    vcur = [vE[2 * b2], vE[2 * b2 + 1]]

                    for h in range(H):
                        pair, row0 = h // 2, (h % 2) * 64
                        sc_t = []
                        for ksrc in (kprev, kcur):
                            ktile, koff = ksrc[pair]
                            scp = p_sc.tile([128, 2, WIN], f32, name="scp", tag="sc")
                            for g in range(2):
                                nc.tensor.matmul(
                                    scp[:, g, :],
                                    ktile[row0:row0 + 64,
                                          koff + g * 128: koff + (g + 1) * 128],
                                    qT[pair][row0:row0 + 64, qsl],
                                    start=True, stop=True)
                            et = work.tile([128, 2, WIN], bf, name="et", tag="exp", bufs=4)
                            nc.scalar.activation(et, scp, AF.Exp)
                            sc_t.append(et)
                        av_ps = p_av.tile([HD + 1, WIN], f32, name="av_ps", tag="av")
                        for ks in range(4):
                            vt = (vprev, vcur)[ks // 2][ks % 2]
                            nc.tensor.matmul(av_ps, vt[:, h, :],
                                             sc_t[ks // 2][:, ks % 2, :],
                                             start=(ks == 0), stop=(ks == 3))
                        rcp = work.tile([1, WIN], f32, name="rcp", tag="rcp", bufs=3)
                        nc.vector.reciprocal(rcp, av_ps[HD:HD + 1, :])
                        rb = work.tile([64, WIN], f32, name="rb", tag="rb", bufs=4)
                        nc.sync.dma_start(out=rb, in_=_rep_ap(rcp, 64))
                        nc.vector.tensor_tensor(
                            out=avT[pair][row0:row0 + 64, qsl],
                            in0=av_ps[0:HD, :], in1=rb, op=op.mult)
                return avT

            def phase_c(i, avT):
                sl = bass.ts(i, CHUNK)
                aT = []
                for dd in range(4):
                    o_ps = p_mm.tile([128, CHUNK], f32, name="o_ps", tag="mm")
                    for c in range(4):
                        nc.tensor.matmul(o_ps, Wo[:, c, bass.ts(dd, 128)],
                                         avT[c], start=(c == 0), stop=(c == 3))
                    at = work.tile([128, CHUNK], bf, name="at", tag="aT", bufs=4)
                    nc.vector.tensor_scalar(out=at, in0=o_ps, scalar1=lb[:, 8 + dd:9 + dd],
                                            scalar2=None, op0=op.add)
                    aT.append(at)

                # Gate/upd matmuls all read the OLD mem; deltas applied after.
                et2s = []
                for dd in range(4):
                    g_ps = p_mm.tile([128, CHUNK], f32, name="g_ps", tag="mm")
                    for c in range(4):
                        nc.tensor.matmul(g_ps, Wg[:, c, bass.ts(dd, 128)],
                                         aT[c], start=(c == 0), stop=False)
                    for c in range(4):
                        nc.tensor.matmul(g_ps, Wg[:, 4 + c, bass.ts(dd, 128)],
                                         memT[c][:, sl], start=False, stop=(c == 3))
                    # sigmoid(z) = 1/(1+exp(-z)): exp on ACT (shared LUT set)
                    ge = work.tile([128, CHUNK], bf, name="ge", tag="gate_e", bufs=2)
                    nc.scalar.activation(ge, g_ps, AF.Exp, scale=-1.0,
                                         bias=lb[:, 20 + dd:21 + dd])
                    nc.vector.tensor_scalar(out=ge, in0=ge, scalar1=1.0,
                                            scalar2=None, op0=op.add)
                    gt = work.tile([128, CHUNK], bf, name="gt", tag="gate", bufs=2)
                    with nc.allow_low_precision(reason="gate in [0,1], bf16 ok"):
                        nc.vector.reciprocal(gt, ge)

                    u_ps = p_mm.tile([128, CHUNK], f32, name="u_ps", tag="mm")
                    for c in range(4):
                        nc.tensor.matmul(u_ps, Wu[:, c, bass.ts(dd, 128)],
                                         aT[c], start=(c == 0), stop=False)
                    for c in range(4):
                        nc.tensor.matmul(u_ps, Wu[:, 4 + c, bass.ts(dd, 128)],
                                         memT[c][:, sl], start=False, stop=(c == 3))
                    dt = work.tile([128, CHUNK], bf, name="dt", tag="tmp", bufs=2)
                    nc.vector.scalar_tensor_tensor(
                        out=dt, in0=u_ps, scalar=lb[:, 16 + dd:17 + dd],
                        in1=memT[dd][:, sl], op0=op.add, op1=op.subtract)
                    et2 = work.tile([128, CHUNK], bf, name="et2", tag="tmp2", bufs=5)
                    nc.vector.tensor_tensor(out=et2, in0=gt, in1=dt, op=op.mult)
                    et2s.append(et2)
                xR = []
                for dd in range(4):
                    nc.vector.tensor_tensor(out=memT[dd][:, sl], in0=memT[dd][:, sl],
                                            in1=et2s[dd], op=op.add)
                    xr = work.tile([128, CHUNK], bf, name="xr", tag="xR", bufs=5)
                    nc.vector.tensor_tensor(out=xr, in0=aT[dd], in1=memT[dd][:, sl],
                                            op=op.add)
                    xR.append(xr)
                _ln_apply(nc, op, p_mm, work, xR,
                          [hT[c][:, sl] for c in range(4)], ones_col, eps_col, CHUNK)

            # Software pipeline: emit B_i, then A_{i+1}, then C_i so the PE
            # stream has independent projection work to chew on while B_i's
            # normalization tail and C_i's LN tail drain on DVE/ACT/DMA.
            st_prev = None if cs == 0 else (prev_k, prev_v)
            st = {cs: phase_a(cs)}
            for i in range(cs, nch):
                avT_i = phase_b(i, st[i], st_prev)
                if i + 1 < nch:
                    st[i + 1] = phase_a(i + 1)
                phase_c(i, avT_i)
                st_prev = (st[i][1], st[i][2])
                del st[i]

            if debug:
                for c in range(4):
                    nc.sync.dma_start(out=d_dbg[f"h{l}"][c], in_=hT[c])
                    nc.sync.dma_start(out=d_dbg[f"m{l}"][c], in_=memT[c])

        wpool.release()

        # ------------------------------------------------------------------
        # temporal conv + double LN + output projection
        # ------------------------------------------------------------------
        fpool = tc.alloc_tile_pool(name="fpool", bufs=1)
        Wconv = fpool.tile([128, CF * 4, 512], bf, name="Wconv")
        nc.sync.dma_start(out=Wconv, in_=d_Wconv.rearrange("j c p d -> p (j c) d"))
        fb = fpool.tile([128, 12], f32, name="fb")
        nc.sync.dma_start(out=fb, in_=d_fbias)
        Wout = fpool.tile([128, 4, F], bf, name="Wout")
        nc.sync.dma_start(out=Wout, in_=d_Wout.rearrange("c p d -> p c d"))
        bout = fpool.tile([F, 1], f32, name="bout")
        nc.sync.dma_start(out=bout, in_=d_bout)

        for oc in range(tout // 512):
            base = HALO + oc * 2048
            cts = []
            for dd in range(4):
                c_ps = p_mm.tile([128, 512], f32, name="c_ps", tag="mm")
                n = 0
                for j in range(CF):
                    for c in range(4):
                        rhs = hT[c][:, base: base + 2048].rearrange(
                            "p (t4 j) -> p t4 j", j=CF)[:, :, j]
                        nc.tensor.matmul(c_ps, Wconv[:, j * 4 + c, bass.ts(dd, 128)],
                                         rhs, start=(n == 0), stop=(n == 15))
                        n += 1
                ct = work.tile([128, 512], bf, name="ct", tag="xR", bufs=5)
                nc.scalar.activation(ct, c_ps, AF.Identity, bias=fb[:, dd:dd + 1])
                cts.append(ct)
            # first LN
            z1 = [fpool.tile([128, 512], bf, name="z1", tag="z1", bufs=4)
                  for _ in range(4)]
            _ln_apply(nc, op, p_mm, work, cts, z1, ones_col, eps_col, 512)
            # cn affine
            c2 = []
            for dd in range(4):
                c2t = fpool.tile([128, 512], bf, name="c2t", tag="c2", bufs=4)
                nc.vector.tensor_scalar(out=c2t, in0=z1[dd],
                                        scalar1=fb[:, 4 + dd:5 + dd],
                                        scalar2=fb[:, 8 + dd:9 + dd],
                                        op0=op.mult, op1=op.add)
                c2.append(c2t)
            # second LN
            z2 = [fpool.tile([128, 512], bf, name="z2", tag="z2", bufs=4)
                  for _ in range(4)]
            _ln_apply(nc, op, p_mm, work, c2, z2, ones_col, eps_col, 512)
            # output projection (on_g/on_b folded into Wout/bout)
            o_ps = p_mm.tile([128, 512], f32, name="o_ps2", tag="mm")
            for c in range(4):
                nc.tensor.matmul(o_ps[0:F, :], Wout[:, c, :], z2[c],
                                 start=(c == 0), stop=(c == 3))
            ot = fpool.tile([F, 512], f32, name="ot", tag="ot", bufs=2)
            nc.scalar.activation(ot, o_ps[0:F, :], AF.Identity, bias=bout)
            nc.sync.dma_start(out=d_outT[:, bass.ts(oc, 512)], in_=ot)
        fpool.release()
        work.release()
        p_av.release()
        p_sc.release()
        p_mm.release()
        state.release()
        const.release()

    nc.compile()
    return nc



def _rep_ap(src, n):
    """Replicate a [1, width] SBUF row AP into [n, width] via a 0-step free dim
    (DMA source); keeps the source's own partition dim/offset encoding."""
    return bass.AP(tensor=src.tensor, offset=src.offset,
                   ap=[list(src.ap[0]), [0, n]] + [list(d) for d in src.ap[1:]])


def _ln_apply(nc, op, p_mm, work, x_tiles, out_targets, ones_col, eps_col, width):
    """LayerNorm (no affine) over 512 channels: out = (x - mean) * rstd.
    x_tiles: 4 sbuf tiles [128, width] bf16; out_targets: 4 APs [128, width].
    Channel sums via ones-vector matmuls; per-token scalars are DMA
    partition-broadcast from the PSUM stat rows, with all scalar math done
    in-place on the broadcast [128, width] tiles (full-lane DVE)."""
    f32 = mybir.dt.float32
    bf = mybir.dt.bfloat16
    st = p_mm.tile([128, width], f32, name="st", tag="mm")
    for c in range(4):
        nc.tensor.matmul(st[0:1, :], ones_col, x_tiles[c],
                         start=(c == 0), stop=(c == 3))
    xsq = []
    for c in range(4):
        xq = work.tile([128, width], bf, name="xq", tag="xsq", bufs=2)
        nc.vector.tensor_tensor(out=xq, in0=x_tiles[c], in1=x_tiles[c], op=op.mult)
        xsq.append(xq)
    for c in range(4):
        nc.tensor.matmul(st[32:33, :], ones_col, xsq[c],
                         start=(c == 0), stop=(c == 3))
    stage = work.tile([33, width], f32, name="stage", tag="stage", bufs=2)
    nc.scalar.copy(stage[0:1, :], st[0:1, :])
    nc.scalar.copy(stage[32:33, :], st[32:33, :])
    bA = work.tile([128, width], f32, name="bA", tag="bA", bufs=2)
    nc.sync.dma_start(out=bA, in_=_rep_ap(stage[0:1, :], 128))
    bB = work.tile([128, width], f32, name="bB", tag="bB", bufs=2)
    nc.sync.dma_start(out=bB, in_=_rep_ap(stage[32:33, :], 128))
    # bA = mean ; bC = mean^2 ; bB = var -> sqrt(var+eps) -> rstd
    nc.vector.tensor_scalar(out=bA, in0=bA, scalar1=1.0 / 512, scalar2=None,
                            op0=op.mult)
    bC = work.tile([128, width], f32, name="bC", tag="bC", bufs=2)
    nc.vector.tensor_tensor(out=bC, in0=bA, in1=bA, op=op.mult)
    nc.vector.scalar_tensor_tensor(out=bB, in0=bB, scalar=1.0 / 512, in1=bC,
                                   op0=op.mult, op1=op.subtract)
    # rstd = exp(-0.5 * ln(var + eps)): keeps ACT in the exp/ln/identity
    # table set (no LoadActFuncSet thrash) and avoids a DVE reciprocal.
    nc.scalar.activation(bB, bB, AF.Ln, bias=eps_col)
    nc.scalar.activation(bB, bB, AF.Exp, scale=-0.5)
    for c in range(4):
        t1 = work.tile([128, width], bf, name="t1", tag="t1", bufs=2)
        nc.vector.tensor_tensor(out=t1, in0=x_tiles[c], in1=bA, op=op.subtract)
        nc.vector.tensor_tensor(out=out_targets[c], in0=t1, in1=bB, op=op.mult)


# ---------------------------------------------------------------------------
# Host-side preprocessing
# ---------------------------------------------------------------------------

def _prep_weights(inp):
    """Fold LN affines / biases / score scale into weights. Returns dict of
    np arrays matching the kernel's DRAM layouts."""
    f = lambda a: np.ascontiguousarray(np.asarray(a), dtype=F32)
    W = {}
    scale = 1.0 / np.sqrt(HD)
    g_prev = np.ones(D, F32)
    b_prev = np.zeros(D, F32)
    Wq_l, Wk_l, Wv_l, Wo_l, Wg_l, Wu_l, lb_l = [], [], [], [], [], [], []
    for l in range(L):
        wq = (g_prev[:, None] * f(inp["Wq"][l])) * scale
        bq = (b_prev @ f(inp["Wq"][l]) + f(inp["bq"][l])) * scale
        wk = g_prev[:, None] * f(inp["Wk"][l])
        bk = b_prev @ f(inp["Wk"][l]) + f(inp["bk"][l])
        wv = g_prev[:, None] * f(inp["Wv"][l])
        bv = b_prev @ f(inp["Wv"][l]) + f(inp["bv"][l])
        wo = f(inp["Wo"][l])
        bo = f(inp["bo"][l]) + bv @ wo
        Wq_l.append(wq.reshape(4, 128, 512))
        Wk_l.append(wk.reshape(4, 128, 512))
        Wv_l.append(wv.reshape(4, 128, 512))
        Wo_l.append(wo.reshape(4, 128, 512))
        Wg_l.append(f(inp["Wg"][l]).reshape(8, 128, 512))
        Wu_l.append(f(inp["Wu"][l]).reshape(8, 128, 512))
        lb = np.zeros((128, 24), F32)
        lb[:, 0:4] = bq.reshape(4, 128).T
        lb[:, 4:8] = bk.reshape(4, 128).T
        lb[:, 8:12] = bo.reshape(4, 128).T
        lb[:, 12:16] = f(inp["bg"][l]).reshape(4, 128).T
        lb[:, 16:20] = f(inp["bu"][l]).reshape(4, 128).T
        lb[:, 20:24] = -f(inp["bg"][l]).reshape(4, 128).T
        lb_l.append(lb)
        g_prev = f(inp["ln_g"][l])
        b_prev = f(inp["ln_b"][l])
    W["Wq"] = np.stack(Wq_l).astype(BF16)
    W["Wk"] = np.stack(Wk_l).astype(BF16)
    W["Wv"] = np.stack(Wv_l).astype(BF16)
    W["Wo"] = np.stack(Wo_l).astype(BF16)
    W["Wg"] = np.stack(Wg_l).astype(BF16)
    W["Wu"] = np.stack(Wu_l).astype(BF16)
    W["lbias"] = np.stack(lb_l)

    W["Wfeat"] = f(inp["W_feat"]).astype(BF16)
    we = f(inp["W_emb"])
    W["We"] = np.stack([we[:FD], we[FD:]]).astype(BF16)
    bemb = f(inp["b_emb"]) + f(inp["b_feat"]) @ we[:FD]
    W["bemb"] = np.ascontiguousarray(bemb.reshape(4, 128).T)

    # conv fold: layer-4 LN affine folded into conv weights
    g4, b4 = g_prev, b_prev
    convw = f(inp["conv_w"])  # [o, c, j]
    wconv = np.zeros((CF, 4, 128, 512), F32)
    for j in range(CF):
        m = (convw[:, :, j] * g4[None, :]).T  # [c, o]
        wconv[j] = m.reshape(4, 128, 512)
    W["Wconv"] = wconv.astype(BF16)
    convb = f(inp["conv_b"]) + np.einsum("c,ocj->o", b4, convw)
    fbias = np.zeros((128, 12), F32)
    fbias[:, 0:4] = convb.reshape(4, 128).T
    fbias[:, 4:8] = f(inp["cn_g"]).reshape(4, 128).T
    fbias[:, 8:12] = f(inp["cn_b"]).reshape(4, 128).T
    W["fbias"] = fbias
    W["Wout"] = np.ascontiguousarray(
        (f(inp["on_g"])[:, None] * f(inp["W_out"])).reshape(4, 128, F)).astype(BF16)
    W["bout"] = np.ascontiguousarray(
        (f(inp["b_out"]) + f(inp["on_b"]) @ f(inp["W_out"])).reshape(F, 1))
    return W


def make_in_maps(inputs, t_tokens=T):
    """Slice full inputs into the 8 per-core input maps."""
    t = t_tokens
    halo = HALO
    own = t - halo
    W = _prep_weights(inputs)
    x = np.asarray(inputs["x"], F32)
    ts = np.asarray(inputs["timestamps"])
    emb = np.asarray(inputs["emb_temp"], F32)

    in_maps = []
    for core in range(N_CORES):
        b, half = core // 2, core % 2
        s0 = half * own
        idx = np.arange(s0 - halo, s0 + own)
        pad = idx < 0
        idx_c = np.clip(idx, 0, S - 1)
        x_sl = x[b][idx_c].copy()
        x_sl[pad] = 0.0
        ts_sl = ts[b][idx_c].copy()
        ts_sl[pad] = 0
        te = emb[ts_sl]                       # [t, FD] host gather
        valid = np.ones(t, F32)
        if half == 0:
            valid[:halo] = 1e-30
        m = dict(W)
        m["xT"] = np.ascontiguousarray(x_sl.T).astype(BF16)
        m["teT"] = np.ascontiguousarray(te.T).astype(BF16)
        m["validP"] = np.ascontiguousarray(valid.reshape(t // 128, 128).T)
        in_maps.append(m)
    return in_maps


_NC_CACHE = {}


def _get_nc():
    if "nc" not in _NC_CACHE:
        _NC_CACHE["nc"] = build_nc()
    return _NC_CACHE["nc"]


def kernel(**inputs):
    nc = _get_nc()
    in_maps = make_in_maps(inputs)
    res = bass_utils.run_bass_kernel_spmd(nc, in_maps, core_ids=list(range(N_CORES)))
    out = np.zeros((B, S // CF, F), F32)
    own4 = (S // 2) // CF
    for core in range(N_CORES):
        b, half = core // 2, core % 2
        o = res.results[core]["outT"]         # [F, TOUT]
        out[b, half * own4:(half + 1) * own4, :] = o.T
    return out

